# revision 80
# baseline (speedup 1.0000x reference)
"""AgentAttention Trainium2 kernel (fp8 DoubleRow edition).

Sharding: data-parallel over batch B=16 across 8 NeuronCores (2 items/core),
no collectives. Per batch item (C=256, N=56*56=3136, 8 heads, hd=32, 49
agents):

  pooling      a[C,49] = 8x8 avg-pool of x           (Pool+DVE reduces)
  agentK/Q     (agent @ wk), (agent @ wq)            (PE bf16, tiny) -> fp8
  v            vpad[C,58x58] & v_nm[N,C]             (PE fp8 DoubleRow on x)
  s1 scores    es1[N,(h,a)] = exp(s*agentK @ x)      (PE fp8 DR + ACT exp)
  agent_v      agv = es1.T @ [v_nm | 1]              (PE fp8 DR; ones-col =
                                                      stage-1 denominators)
  s2 scores    w2T[(h-pair,a),N] = exp(s*agentQ @ x) (PE fp8 DR + ACT exp)
  s2 denoms    ones @ w2T -> Ln -> bc4-matmul bcast  (PE + ACT, fp16)
               -> exp(-x) = 1/s2 broadcast to rows   (no DRAM bounce)
  u            u = av_ext.T @ w2T                    (PE bf16 x fp8)
  dwc          9 bf16 diag-matmul taps over contiguous 462-wide windows
               (bf16 v via a dedicated bf16 v-proj: fp8 noise in the dwc
               chain does not average out and would bust the 2e-2 budget)
  combine      sum = (u * rbc) + (dwc_psum + dwc_b)  (DVE)
  out          proj_w @ sum + proj_b + x             (PE bf16 + DVE)

Per-item tiles are double-buffered and emission is software-pipelined
(head = attention/ACT-heavy, tail = dwc+proj/PE-heavy, interleaved per
channel-group) so consecutive items overlap across engines. Matmuls whose
stationaries load at different PE row bases never share a PSUM bank (HW
restriction found empirically), and each PSUM accumulation chain owns its
2KB zero region.
"""

import sys

for _p in ("/opt/trn_rl_repo", "/opt/trn_rl_repo/concourse"):
    if _p not in sys.path:
        sys.path.insert(0, _p)

import numpy as np
import ml_dtypes

import concourse.bass as bass
import concourse.bacc as bacc
import concourse.mybir as mybir
import concourse.tile as tile
from concourse.bass_utils import run_bass_kernel_spmd

BF16 = ml_dtypes.bfloat16
FP16 = np.float16
F8NP = ml_dtypes.float8_e4m3
FP32 = np.float32


class _Bacc(bacc.Bacc):
    """Bacc whose activation-table pass only sees tables that serve every
    function this kernel uses, so the greedy per-function table choice
    cannot alternate between exp-only and ln-only sets. Table ids keep
    their global act_info.json indices."""

    def insert_act_table_loads(self):
        import concourse.mybir as _mb
        from concourse.hw_specs import get_activation_tables
        import bass_rust as _bass_rust

        acts = [
            i
            for b in self.main_func.blocks
            for i in b.instructions
            if isinstance(i, _mb.InstActivation)
        ]
        if not acts:
            return
        needed = {i.func for i in acts}
        tables = list(get_activation_tables(self.m.arch).items())
        assert any(needed <= funcs for _, funcs in tables), (
            f"no single activation table covers {needed}")
        filtered = [
            (name, funcs if needed <= funcs else set())
            for name, funcs in tables
        ]
        _bass_rust.insert_act_table_loads(self, filtered)


B, C, HH, WW = 16, 256, 56, 56
N = HH * WW            # 3136
NH, HD, A = 8, 32, 49
SCALE = float(HD) ** -0.5
NCORES = 8
BPC = B // NCORES      # 2
CT = 2                 # 128-channel tiles
NP = 25                # ceil(N/128); last chunk is 64
FCH = 448              # free-dim chunk = 8 image rows
NF = 7
PAD = 58
IMG = PAD * PAD        # 3364
VPW = IMG
BD = mybir.dt.bfloat16
FD = mybir.dt.float32
HD16 = mybir.dt.float16
F8 = mybir.dt.float8e4
DRM = mybir.MatmulPerfMode.DoubleRow
FX = mybir.ActivationFunctionType
OP = mybir.AluOpType

# dwc taps (dy, dx), tap index ti = 3*dy + dx, vpad offset = 58*dy + dx.
# All 9 taps run as bf16 diag matmuls over contiguous 462-wide windows
# (fp8 anywhere in the dwc chain costs ~3-4% of the dwc contribution,
# which alone busts the 2e-2 budget); the bias rides the combine's
# scalar_tensor_tensor.
TAPS = [(dy, dx) for dy in range(3) for dx in range(3)]


def _pchunk(i):
    n0 = 128 * i
    return n0, min(128, N - n0)


def _grp(n, size):
    return [list(range(s, min(s + size, n))) for s in range(0, n, size)]


def build_bass(reps=1):
    nc = _Bacc()
    d = {}
    d["xbf"] = nc.declare_dram_parameter("xbf", [BPC, C, N], BD,
                                         isOutput=False)
    d["xq8"] = nc.declare_dram_parameter("xq8", [BPC, C, N], F8,
                                         isOutput=False)
    d["wv8"] = nc.declare_dram_parameter("wv8", [128, 2, C], F8,
                                         isOutput=False)
    d["wkci"] = nc.declare_dram_parameter("wkci", [CT, 128, 2, 128], BD,
                                          isOutput=False)
    d["wqci"] = nc.declare_dram_parameter("wqci", [CT, 128, 2, 128], BD,
                                          isOutput=False)
    d["wprojT"] = nc.declare_dram_parameter("wprojT", [C, C], BD,
                                            isOutput=False)
    d["projb"] = nc.declare_dram_parameter("projb", [C, 1], FD,
                                           isOutput=False)
    d["wvbf"] = nc.declare_dram_parameter("wvbf", [C, C], BD, isOutput=False)
    d["dwcdiag"] = nc.declare_dram_parameter("dwcdiag", [9, CT, 128, 128],
                                             BD, isOutput=False)
    d["dwcb"] = nc.declare_dram_parameter("dwcb", [C, 1], FD, isOutput=False)
    d["bc4"] = nc.declare_dram_parameter("bc4", [128, 128], HD16,
                                         isOutput=False)
    d["out32"] = nc.declare_dram_parameter("out32", [BPC, C, N], BD,
                                           isOutput=True)
    with tile.TileContext(nc) as tc:
        _emit(nc, tc, d, reps)
    nc.finalize()
    return nc


def _emit(nc, tc, d, reps=1):
    import contextlib
    ctx = contextlib.ExitStack()
    with ctx:
        persist = ctx.enter_context(tc.tile_pool(name="persist", bufs=1))
        small = ctx.enter_context(tc.tile_pool(name="small", bufs=2))
        upch = ctx.enter_context(tc.tile_pool(name="upch", bufs=3))
        xbfp = ctx.enter_context(tc.tile_pool(name="xbfp", bufs=2))
        x8p = ctx.enter_context(tc.tile_pool(name="x8p", bufs=2))
        ostp = ctx.enter_context(tc.tile_pool(name="ostp", bufs=2))
        # per-item double-buffered tiles so item k+1's ACT-heavy attention
        # phase can run under item k's PE-heavy dwc/proj phase
        item = ctx.enter_context(tc.tile_pool(name="item", bufs=2))
        # PSUM: "big" 2-bank tiles x2 + "pk" 1-bank tiles x4 = 8 banks
        psum = ctx.enter_context(tc.tile_pool(name="psum", bufs=1,
                                              space="PSUM"))

        t = {}
        t["wv8"] = persist.tile([128, 2, C], F8, name="wv8")
        t["wkci"] = persist.tile([128, CT, 2, 128], BD, name="wkci")
        t["wqci"] = persist.tile([128, CT, 2, 128], BD, name="wqci")
        t["wproj"] = persist.tile([128, 2, C], BD, name="wproj")
        t["projb"] = persist.tile([128, 2, 1], FD, name="projb")
        t["wvbf"] = persist.tile([128, 2, C], BD, name="wvbf")
        t["dwcdiag"] = persist.tile([128, 9, CT, 128], BD, name="dwcdiag")
        t["dwcb"] = persist.tile([128, 2, 1], FD, name="dwcb")
        t["bc4"] = persist.tile([128, 128], HD16, name="bc4")
        t["aT"] = persist.tile([128, CT, A], FD, name="aT")
        t["agBD1"] = persist.tile([128, CT, 4 * A], BD, name="agBD1")
        t["agBD2"] = persist.tile([128, CT, 113], BD, name="agBD2")
        t["onesBD"] = persist.tile([128, 34], BD, name="onesBD")
        t["sum"] = persist.tile([128, CT, N], BD, name="sum")

        nc.vector.memset(t["agBD1"][:, :, :], 0.0)
        nc.vector.memset(t["agBD2"][:, :, :], 0.0)
        # onesBD col 0/32: head-even denom (q=0/1); col 1/33: head-odd;
        # cols 2:32 produce dummy positive sums so one Ln can span rows
        # 0:34 without hitting stale (possibly negative) psum
        nc.vector.memset(t["onesBD"][:, :], 0.0)
        nc.vector.memset(t["onesBD"][0:A, 0:1], 1.0)
        nc.vector.memset(t["onesBD"][0:A, 2:33], 1.0)
        nc.vector.memset(t["onesBD"][64:113, 1:2], 1.0)
        nc.vector.memset(t["onesBD"][64:113, 33:34], 1.0)
        # chunk-24 garbage rows of v_nm/es1 are never read (K=64 there)

        nc.sync.dma_start(out=t["wv8"][:, :, :], in_=d["wv8"][:, :, :])
        nc.sync.dma_start(out=t["wkci"][:, :, :, :],
                          in_=d["wkci"].rearrange("g p c m -> p g c m"))
        nc.sync.dma_start(out=t["wqci"][:, :, :, :],
                          in_=d["wqci"].rearrange("g p c m -> p g c m"))
        nc.sync.dma_start(out=t["wproj"][:, :, :],
                          in_=d["wprojT"].rearrange("(t p) f -> p t f", p=128))
        nc.sync.dma_start(out=t["projb"][:, :, :],
                          in_=d["projb"].rearrange("(t p) o -> p t o", p=128))
        nc.sync.dma_start(out=t["wvbf"][:, :, :],
                          in_=d["wvbf"].rearrange("(t p) f -> p t f", p=128))
        nc.sync.dma_start(out=t["dwcdiag"][:, :, :, :],
                          in_=d["dwcdiag"].rearrange("k g p f -> p k g f"))
        nc.sync.dma_start(out=t["dwcb"][:, :, :],
                          in_=d["dwcb"].rearrange("(t p) o -> p t o", p=128))
        nc.sync.dma_start(out=t["bc4"][:, :], in_=d["bc4"][:, :])

        lncp = ctx.enter_context(tc.tile_pool(name="lncp", bufs=2))
        pools = dict(psum=psum, small=small, upch=upch, xbfp=xbfp,
                     x8p=x8p, ostp=ostp, item=item, lncp=lncp,
                     lnc_init=[0])
        idx = 0
        for _ in range(reps):
            for b in range(BPC):
                _emit_item(nc, tc, b, d, t, pools, idx)
                idx += 1


def _big(pools, nm):
    return pools["psum"].tile([128, 1024], FD, name=nm, tag="pbig", bufs=3)


def _bank(pools, nm):
    return pools["psum"].tile([128, 512], FD, name=nm, tag="pk", bufs=2)


DEBUG_PHASE = 99   # 0=loads 1=+pool 2=+preps 3=+s1/v/agv 4=+stage2 5=+dwc


def _dbg_out(nc, b, d, xbf):
    for mt in range(CT):
        nc.gpsimd.dma_start(
            out=d["out32"][b, 128 * mt:128 * (mt + 1), :],
            in_=xbf[:, mt, :])


def _emit_item(nc, tc, b, d, t, pools, idx):
    it = pools["item"]
    t = dict(t)
    t["agK8"] = it.tile([128, 2, 392], F8, name="agK8", tag="agK8", bufs=2)
    t["agQ8"] = it.tile([128, 2, 4, 128], F8, name="agQ8", tag="agQ8",
                        bufs=2)
    t["vpad"] = it.tile([128, CT, VPW], BD, name="vpad", tag="vpad", bufs=2)
    t["v_nm"] = it.tile([128, NP, NH, 33], F8, name="v_nm", tag="v_nm",
                        bufs=2)
    t["es1"] = it.tile([128, NP, NH, 64], F8, name="es1", tag="es1", bufs=2)
    t["w2T"] = it.tile([128, 4, N], F8, name="w2T", tag="w2T", bufs=2)
    t["av_ext"] = it.tile([128, 4, 66], BD, name="av_ext", tag="av_ext",
                          bufs=2)
    t["rtile"] = it.tile([128, 8], FD, name="rtile", tag="rtile", bufs=2)
    t["rbc"] = it.tile([128, CT, N], BD, name="rbc", tag="rbc", bufs=2)
    if idx < 2:
        # per-buffer init of regions later matmuls read but no per-item
        # pass rewrites (each of the two rotating buffers sees this once)
        nc.gpsimd.memset(t["es1"][:, :, :, A:64], 0.0)       # head pads
        nc.gpsimd.memset(t["v_nm"][:, :, :, 32:33], 1.0)     # ones cols
        nc.gpsimd.memset(t["vpad"][:, :, 0:PAD], 0.0)        # top border
        nc.gpsimd.memset(t["vpad"][:, :, 57 * PAD:58 * PAD], 0.0)
        nc.gpsimd.memset(
            t["vpad"][:, :, 0:IMG].rearrange("p g (r c) -> p g r c", c=PAD)
            [:, :, 1:57, 0:1], 0.0)                          # left border
        nc.gpsimd.memset(
            t["vpad"][:, :, 0:IMG].rearrange("p g (r c) -> p g r c", c=PAD)
            [:, :, 1:57, 57:58], 0.0)                        # right border
        nc.vector.memset(t["av_ext"][:, :, :], 0.0)
        nc.vector.memset(t["av_ext"][0:A, :, 64:65], 1.0)    # s2 denoms
        nc.vector.memset(t["av_ext"][64:113, :, 65:66], 1.0)
        nc.gpsimd.memset(t["agQ8"][:, :, :, :], 0.0)   # pad cols 113:128

    # ---------------- load x ----------------
    xbf = pools["xbfp"].tile([128, CT, N], BD, name="xbf", tag="xbf", bufs=2)
    xq8 = pools["x8p"].tile([128, CT, N], F8, name="xq8", tag="x8", bufs=2)
    for g in range(CT):
        nc.sync.dma_start(out=xbf[:, g, :],
                          in_=d["xbf"][b, 128 * g:128 * (g + 1), :])
        nc.scalar.dma_start(out=xq8[:, g, :],
                            in_=d["xq8"][b, 128 * g:128 * (g + 1), :])

    if DEBUG_PHASE <= 0:
        return _dbg_out(nc, b, d, xbf)

    # ------- pooling -> agent blocks (8-way add trees on Pool engine) -----
    for g in range(CT):
        a1 = pools["small"].tile([128, 56, 7], FD, name="a1", tag="a1")
        xv = xbf[:, g, :].rearrange("p (h wb wi) -> p h wb wi", wb=7, wi=8)
        nc.gpsimd.tensor_tensor(out=a1[:, :, :], in0=xv[:, :, :, 0],
                                in1=xv[:, :, :, 1], op=OP.add)
        for k in range(2, 8):
            nc.gpsimd.tensor_tensor(out=a1[:, :, :], in0=a1[:, :, :],
                                    in1=xv[:, :, :, k], op=OP.add)
        a1r = a1[:, :, :].rearrange("p (hb hi) wb -> p hb hi wb", hi=8)
        av = t["aT"][:, g, :].rearrange("p (hb wb) -> p hb wb", wb=7)
        nc.gpsimd.tensor_tensor(out=av, in0=a1r[:, :, 0, :],
                                in1=a1r[:, :, 1, :], op=OP.add)
        for k in range(2, 8):
            nc.gpsimd.tensor_tensor(out=av, in0=av, in1=a1r[:, :, k, :],
                                    op=OP.add)
        # the two 8-sums fold into one /64; scale applied during the copies
        for hp in range(4):
            nc.vector.tensor_scalar_mul(
                out=t["agBD1"][32 * hp:32 * (hp + 1), g, A * hp:A * (hp + 1)],
                in0=t["aT"][32 * hp:32 * (hp + 1), g, :], scalar1=1.0 / 64.0)
        for q in range(2):
            nc.vector.tensor_scalar_mul(
                out=t["agBD2"][64 * q:64 * q + 32, g, 0:A],
                in0=t["aT"][64 * q:64 * q + 32, g, :], scalar1=1.0 / 64.0)
            nc.vector.tensor_scalar_mul(
                out=t["agBD2"][64 * q + 32:64 * q + 64, g, 64:113],
                in0=t["aT"][64 * q + 32:64 * q + 64, g, :], scalar1=1.0 / 64.0)

    if DEBUG_PHASE <= 1:
        return _dbg_out(nc, b, d, xbf)

    # ------------- agentK / agentQ: fold wk, wq into the agents ----------
    # agK[ci, (h,a)] = sum_co wk[co, ci] * agent_bd1[co, (h,a)]
    # (per co-tile g, the 196-wide block of heads 4g..4g+3)
    for c in range(2):
        pk_ = _bank(pools, "pprep")
        for g in range(CT):
            nc.tensor.matmul(pk_[:, 196 * g:196 * (g + 1)],
                             t["wkci"][:, g, c, :], t["agBD1"][:, g, :],
                             start=True, stop=True)
        with nc.allow_low_precision("fp8 agent-key weights"):
            nc.vector.tensor_copy(out=t["agK8"][:, c, :], in_=pk_[:, 0:392])
    # matmuls whose stationaries load at different PE row bases (q=0 vs
    # q=1) must not share a psum bank -> one bank per q
    for c in range(2):
        pqs = [_bank(pools, "pprepA"), _bank(pools, "pprepB")]
        for s in range(4):
            g, q = divmod(s, 2)
            nc.tensor.matmul(pqs[q][:, 128 * g:128 * g + 113],
                             t["wqci"][64 * q:64 * (q + 1), g, c, :],
                             t["agBD2"][64 * q:64 * (q + 1), g, :],
                             start=True, stop=True)
        with nc.allow_low_precision("fp8 agent-query weights"):
            for q in range(2):
                dst = (t["agQ8"][:, c, :, :]
                       .rearrange("p (g qq) m -> p qq g m", qq=2)
                       [:, q, :, 0:113])
                nc.vector.tensor_copy(
                    out=dst,
                    in_=pqs[q][:, 0:256]
                    .rearrange("p (g m) -> p g m", m=128)[:, :, 0:113])

    if DEBUG_PHASE <= 2:
        return _dbg_out(nc, b, d, xbf)

    # ---- stage-1 scores/exp + v_nm + agent_v ----

    def s1_group(iis):
        pt = _big(pools, "p1")
        i0 = iis[0]
        for i in iis:
            n0, sz = _pchunk(i)
            nc.tensor.matmul(
                pt[0:sz, 512 * (i - i0):512 * (i - i0) + 392],
                xq8[:, :, n0:n0 + sz], t["agK8"][:, :, :],
                start=True, stop=True, perf_mode=DRM)
        full = [i for i in iis if _pchunk(i)[1] == 128]
        with nc.allow_low_precision("fp8 attention weights"):
            if full:
                nfull = len(full)
                src = (pt[:, 512 * (full[0] - i0):
                          512 * (full[0] - i0) + 512 * nfull]
                       .rearrange("p (j f) -> p j f", f=512)[:, :, 0:392]
                       .rearrange("p j (h a) -> p j h a", a=A))
                nc.scalar.activation(
                    out=t["es1"][:, full[0]:full[0] + nfull, :, 0:A],
                    in_=src, func=FX.Exp, scale=SCALE)
            for i in iis:
                n0, sz = _pchunk(i)
                if sz == 128:
                    continue
                src = (pt[0:sz, 512 * (i - i0):512 * (i - i0) + 392]
                       .rearrange("p (h a) -> p h a", a=A))
                nc.scalar.activation(
                    out=t["es1"][0:sz, i, :, 0:A],
                    in_=src, func=FX.Exp, scale=SCALE)

    def v_group(iis):
        pt = _big(pools, "pv")
        i0, nj = iis[0], len(iis)
        for i in iis:
            n0, sz = _pchunk(i)
            nc.tensor.matmul(
                pt[0:sz, 512 * (i - i0):512 * (i - i0) + C],
                xq8[:, :, n0:n0 + sz], t["wv8"][:, :, :],
                start=True, stop=True, perf_mode=DRM)
        rows = min(_pchunk(i)[1] for i in iis)
        src = (pt[0:rows, 0:512 * nj].rearrange("p (j f) -> p j f", f=512)
               [:, :, 0:C].rearrange("p j (h dd) -> p j h dd", dd=32))
        if (i0 // 2) % 2 == 0:
            nc.scalar.copy(out=t["v_nm"][0:rows, i0:i0 + nj, :, 0:32],
                           in_=src)
        else:
            with nc.allow_low_precision("fp8 values"):
                nc.vector.tensor_copy(
                    out=t["v_nm"][0:rows, i0:i0 + nj, :, 0:32], in_=src)

    def agv_pair(p):
        # one accumulation chain (own psum bank/zero-region) per head-pair;
        # pairs of full chunks ride one DoubleRow matmul, the 64-row tail
        # chunk is a plain fp8 matmul
        bank = _bank(pools, "agv%d" % p)
        for i in range(0, NP - 1, 2):
            nc.tensor.matmul(
                bank[:, 0:66],
                t["es1"][:, i:i + 2, 2 * p:2 * p + 2, :].rearrange(
                    "p i h dd -> p i (h dd)"),
                t["v_nm"][:, i:i + 2, 2 * p:2 * p + 2, :].rearrange(
                    "p i h dd -> p i (h dd)"),
                start=(i == 0), stop=False, perf_mode=DRM)
        i, (n0, sz) = NP - 1, _pchunk(NP - 1)
        nc.tensor.matmul(
            bank[:, 0:66],
            t["es1"][0:sz, i, 2 * p:2 * p + 2, :].rearrange(
                "p h dd -> p (h dd)"),
            t["v_nm"][0:sz, i, 2 * p:2 * p + 2, :].rearrange(
                "p h dd -> p (h dd)"),
            start=False, stop=True)
        return bank

    def vpad_group(g, js):
        # bf16 v for the dwc path (fp8 v noise does not average out of the
        # depthwise conv and would alone exceed the error budget)
        pt = _big(pools, "pvp")
        j0, nj = js[0], len(js)
        for j in js:
            for kt in range(CT):
                nc.tensor.matmul(
                    pt[:, 512 * (j - j0):512 * (j - j0) + FCH],
                    t["wvbf"][:, kt, 128 * g:128 * (g + 1)],
                    xbf[:, kt, FCH * j:FCH * (j + 1)],
                    start=(kt == 0), stop=(kt == CT - 1))
        dst = (t["vpad"][:, g, 0:IMG]
               .rearrange("p (r c) -> p r c", c=PAD)
               [:, 1 + 8 * j0:1 + 8 * (j0 + nj), 1:57]
               .rearrange("p (j r) c -> p j r c", r=8))
        src = (pt[:, 0:512 * nj].rearrange("p (j f) -> p j f", f=512)
               [:, :, 0:FCH].rearrange("p j (r c) -> p j r c", c=56))
        with nc.allow_low_precision("bf16 values"):
            nc.vector.tensor_copy(out=dst, in_=src)

    groups = _grp(NP, 2)
    vjobs = [(g, js) for g in range(CT) for js in _grp(NF, 2)]
    for k, iis in enumerate(groups):
        s1_group(iis)
        v_group(iis)
        if k < len(vjobs):
            vpad_group(*vjobs[k])
    for g, js in vjobs[len(groups):]:
        vpad_group(g, js)

    for p in range(4):
        bank = agv_pair(p)
        nc.vector.reciprocal(out=t["rtile"][0:A, p:p + 1],
                             in_=bank[0:A, 32:33])
        nc.vector.reciprocal(out=t["rtile"][64:113, 4 + p:5 + p],
                             in_=bank[64:113, 65:66])
        with nc.allow_low_precision("bf16 attention weights"):
            nc.vector.tensor_tensor(
                out=t["av_ext"][0:A, p, 0:32], in0=bank[0:A, 0:32],
                in1=t["rtile"][0:A, p:p + 1].to_broadcast([A, 32]),
                op=OP.mult)
            nc.vector.tensor_tensor(
                out=t["av_ext"][64:113, p, 32:64], in0=bank[64:113, 33:65],
                in1=t["rtile"][64:113, 4 + p:5 + p].to_broadcast([A, 32]),
                op=OP.mult)

    if DEBUG_PHASE <= 3:
        return _dbg_out(nc, b, d, xbf)

    # ------- stage 2: scores/exp + denominator chains ------
    for s in range(4):
        for js in _grp(NF, 2):
            j0, nj = js[0], len(js)
            pt = _big(pools, "p2")
            for j in js:
                nc.tensor.matmul(
                    pt[:, 512 * (j - j0):512 * (j - j0) + FCH],
                    t["agQ8"][:, :, s, :],
                    xq8[:, :, FCH * j:FCH * (j + 1)],
                    start=True, stop=True, perf_mode=DRM)
            src = (pt[0:113, 0:512 * nj]
                   .rearrange("p (j f) -> p j f", f=512)[:, :, 0:FCH])
            dst = (t["w2T"][0:113, s, :]
                   .rearrange("p (j f) -> p j f", f=FCH)
                   [:, j0:j0 + nj, :])
            nc.scalar.activation(out=dst, in_=src, func=FX.Exp,
                                 scale=SCALE)
    for a in range(2):
        # denominators: two ones-matmuls per chunk put head 4a+2q+e's
        # denominator at psum row 32q+e (rows 2:32 get dummy positive sums);
        # ONE Ln per chunk-group covers all four heads; bc4 broadcasts each
        # lns2 row to its head's 32 channel rows; Exp(-x) = 1/s2.
        for js in _grp(NF, 2):
            j0, nj = js[0], len(js)
            pt = _big(pools, "po")
            for j in js:
                nc.tensor.matmul(
                    pt[0:32, 512 * (j - j0):512 * (j - j0) + FCH],
                    t["onesBD"][0:113, 0:32],
                    t["w2T"][0:113, 2 * a, FCH * j:FCH * (j + 1)],
                    start=True, stop=True)
                nc.tensor.matmul(
                    pt[32:34, 512 * (j - j0):512 * (j - j0) + FCH],
                    t["onesBD"][0:113, 32:34],
                    t["w2T"][0:113, 2 * a + 1, FCH * j:FCH * (j + 1)],
                    start=True, stop=True)
            lnc = pools["lncp"].tile([128, 2, FCH], HD16, name="lnc",
                                     tag="lnc", bufs=2)
            src = (pt[0:34, 0:512 * nj]
                   .rearrange("p (j f) -> p j f", f=512)[:, :, 0:FCH])
            with nc.allow_low_precision("fp16 log-denominators"):
                nc.scalar.activation(out=lnc[0:34, 0:nj, :], in_=src,
                                     func=FX.Ln)
            pb = _big(pools, "pb")
            for j in js:
                nc.tensor.matmul(
                    pb[:, 512 * (j - j0):512 * (j - j0) + FCH],
                    t["bc4"][0:34, :], lnc[0:34, j - j0, :],
                    start=True, stop=True)
            bsrc = (pb[:, 0:512 * nj].rearrange("p (j f) -> p j f", f=512)
                    [:, :, 0:FCH])
            dst = (t["rbc"][:, a, :].rearrange("p (j f) -> p j f", f=FCH)
                   [:, j0:j0 + nj, :])
            with nc.allow_low_precision("bf16 softmax reciprocal"):
                nc.scalar.activation(out=dst, in_=bsrc, func=FX.Exp,
                                     scale=-1.0)

    if DEBUG_PHASE <= 4:
        return _dbg_out(nc, b, d, xbf)

    # ---------------- dwc + apply + combine ----------------
    # dwc: 5 fp8-DR diag-pair matmuls per (g, j); the last pair is
    # (tap 8, bias) with its second k-tile reading vpad's ones region.
    # moving operand per (tap, j): contiguous 462-wide window over 8 padded
    # rows starting at the tap offset. Output position f = 58*r + c maps to
    # image pixel (8j+r, c); f % 58 in {56, 57} is junk never read back.
    for g in range(2):
        vpg = t["vpad"][:, g, :]
        for half in _grp(NF, 2):
            pds = [_big(pools, "pd") for _ in range((len(half) + 1) // 2)]
            for k in range(9):
                dy, dx = TAPS[k]
                for idx, j in enumerate(half):
                    off1 = 58 * (dy + 8 * j) + dx
                    mv = bass.AP(
                        tensor=vpg.tensor,
                        offset=vpg.offset + off1,
                        ap=[[CT * VPW, 128], [1, 462]])
                    nc.tensor.matmul(
                        pds[idx // 2][:, 512 * (idx % 2):512 * (idx % 2)
                                      + 462],
                        t["dwcdiag"][:, k, g, :], mv,
                        start=(k == 0), stop=(k == 8))
            pas = [_bank(pools, "pa") for _ in half]
            for q in range(2):
                for idx, j in enumerate(half):
                    nc.tensor.matmul(
                        pas[idx][64 * q:64 * (q + 1), 0:FCH],
                        t["av_ext"][0:113, 2 * g + q, 0:64],
                        t["w2T"][0:113, 2 * g + q, FCH * j:FCH * (j + 1)],
                        start=True, stop=True)
            for idx, j in enumerate(half):
                pd_ap = pds[idx // 2][:, :]
                pd = bass.AP(tensor=pd_ap.tensor,
                             offset=pd_ap.offset + 512 * (idx % 2),
                             ap=[[1024, 128], [58, 8], [1, 56]])
                pa = pas[idx]
                upt = pools["upch"].tile([128, FCH], BD, name="up", tag="up")
                with nc.allow_low_precision("bf16 activations"):
                    # u' = u * (1/s2)          (DVE, PSUM read)
                    nc.vector.tensor_tensor(
                        out=upt[:, :], in0=pa[:, 0:FCH],
                        in1=t["rbc"][:, g, FCH * j:FCH * (j + 1)], op=OP.mult)
                    # sum = (dwc_psum + dwc_b) + u'   (DVE, PSUM read)
                    nc.vector.scalar_tensor_tensor(
                        out=t["sum"][:, g, FCH * j:FCH * (j + 1)]
                        .rearrange("p (r c) -> p r c", c=56),
                        in0=pd, scalar=t["dwcb"][:, g, :],
                        in1=upt[:, :].rearrange("p (r c) -> p r c", c=56),
                        op0=OP.add, op1=OP.add)

    if DEBUG_PHASE <= 5:
        return _dbg_out(nc, b, d, xbf)

    # ---------------- proj + bias + residual ----------------
    for mt in range(CT):
        for js in _grp(NF, 2):
            j0, nj = js[0], len(js)
            pp = _big(pools, "pp")
            for kt in range(CT):
                for j in js:
                    nc.tensor.matmul(
                        pp[:, 512 * (j - j0):512 * (j - j0) + FCH],
                        t["wproj"][:, kt, 128 * mt:128 * (mt + 1)],
                        t["sum"][:, kt, FCH * j:FCH * (j + 1)],
                        start=(kt == 0), stop=(kt == CT - 1))
            src = (pp[:, 0:512 * nj].rearrange("p (j f) -> p j f", f=512)
                   [:, :, 0:FCH])
            ost = pools["ostp"].tile([128, 2, FCH], BD, name="ost",
                                     tag="ost", bufs=2)
            resid = (xbf[:, mt, FCH * j0:FCH * (j0 + nj)]
                     .rearrange("p (j f) -> p j f", f=FCH))
            with nc.allow_low_precision("bf16 output"):
                nc.vector.scalar_tensor_tensor(
                    out=ost[:, 0:nj, :], in0=src, scalar=t["projb"][:, mt, :],
                    in1=resid, op0=OP.add, op1=OP.add)
            nc.gpsimd.dma_start(
                out=d["out32"][b, 128 * mt:128 * (mt + 1),
                               FCH * j0:FCH * (j0 + nj)],
                in_=ost[:, 0:nj, :])


def host_prep(x, qkv_w, proj_w, proj_b, dwc_w, dwc_b):
    xf = np.ascontiguousarray(x.reshape(B, C, N), dtype=FP32)
    qkv = np.asarray(qkv_w, FP32)
    wv8 = np.zeros((128, 2, C), FP32)
    for kt in range(2):
        # wv8[p, kt, co] = qkv_w[512+co, 128*kt+p]
        wv8[:, kt, :] = qkv[512:768, 128 * kt:128 * (kt + 1)].T
    wv8 = np.ascontiguousarray(wv8.astype(F8NP))
    wkci = np.zeros((CT, 128, 2, 128), FP32)
    wqci = np.zeros((CT, 128, 2, 128), FP32)
    for g in range(CT):
        for c in range(2):
            # wkci[g, p, c, m] = wk[128g+p, 128c+m]
            wkci[g, :, c, :] = qkv[256 + 128 * g:256 + 128 * (g + 1),
                                   128 * c:128 * (c + 1)]
            # wqci[g, 64q+p, c, m] = wq[64*(2g+q)+p, 128c+m]
            for q in range(2):
                s = 2 * g + q
                wqci[g, 64 * q:64 * (q + 1), c, :] = (
                    qkv[64 * s:64 * (s + 1), 128 * c:128 * (c + 1)])
    wkci = np.ascontiguousarray(wkci.astype(BF16))
    wqci = np.ascontiguousarray(wqci.astype(BF16))
    wprojT = np.ascontiguousarray(np.asarray(proj_w, FP32).T.astype(BF16))
    wvbfT = np.ascontiguousarray(qkv[512:768].T.astype(BF16))
    projb = np.ascontiguousarray(np.asarray(proj_b, FP32).reshape(C, 1))
    dwcb = np.ascontiguousarray(np.asarray(dwc_b, FP32).reshape(C, 1))
    w33 = np.asarray(dwc_w, FP32).reshape(C, 9)
    dd = np.zeros((9, CT, 128, 128), FP32)
    for k in range(9):
        for g in range(CT):
            np.fill_diagonal(dd[k, g], w33[128 * g:128 * (g + 1), k])
    dwcdiag = np.ascontiguousarray(dd.astype(BF16))
    bc4 = np.zeros((128, 128), FP32)
    for e, row in enumerate((0, 1, 32, 33)):
        bc4[row, 32 * e:32 * (e + 1)] = 1.0
    bc4 = np.ascontiguousarray(bc4.astype(FP16))
    maps = []
    for c in range(NCORES):
        xs = xf[BPC * c:BPC * (c + 1)]
        maps.append(dict(
            xbf=np.ascontiguousarray(xs.astype(BF16)),
            xq8=np.ascontiguousarray(xs.astype(F8NP)),
            wv8=wv8, wkci=wkci, wqci=wqci, wprojT=wprojT, wvbf=wvbfT,
            projb=projb, dwcdiag=dwcdiag, dwcb=dwcb, bc4=bc4,
        ))
    return maps


_NC_CACHE = {}


def kernel(x, qkv_w, proj_w, proj_b, dwc_w, dwc_b, trace=False):
    if "nc" not in _NC_CACHE:
        _NC_CACHE["nc"] = build_bass()
    nc = _NC_CACHE["nc"]
    maps = host_prep(np.asarray(x), np.asarray(qkv_w), np.asarray(proj_w),
                     np.asarray(proj_b), np.asarray(dwc_w), np.asarray(dwc_b))
    res = run_bass_kernel_spmd(nc, maps, core_ids=list(range(NCORES)),
                               trace=trace)
    outs = [np.asarray(r["out32"]).astype(np.float32).reshape(BPC, C, HH, WW)
            for r in res.results]
    full = np.concatenate(outs, axis=0)
    if trace:
        return full, res
    return full


# revision 83
# speedup vs baseline: 1.0219x; 1.0219x over previous
"""AgentAttention Trainium2 kernel (fp8 DoubleRow edition).

Sharding: data-parallel over batch B=16 across 8 NeuronCores (2 items/core),
no collectives. Per batch item (C=256, N=56*56=3136, 8 heads, hd=32, 49
agents):

  pooling      a[C,49] = 8x8 avg-pool of x           (Pool+DVE reduces)
  agentK/Q     (agent @ wk), (agent @ wq)            (PE bf16, tiny) -> fp8
  v            vpad[C,58x58] & v_nm[N,C]             (PE fp8 DoubleRow on x)
  s1 scores    es1[N,(h,a)] = exp(s*agentK @ x)      (PE fp8 DR + ACT exp)
  agent_v      agv = es1.T @ [v_nm | 1]              (PE fp8 DR; ones-col =
                                                      stage-1 denominators)
  s2 scores    w2T[(h-pair,a),N] = exp(s*agentQ @ x) (PE fp8 DR + ACT exp)
  s2 denoms    ones @ w2T -> Ln -> bc4-matmul bcast  (PE + ACT, fp16)
               -> exp(-x) = 1/s2 broadcast to rows   (no DRAM bounce)
  u            u = av_ext.T @ w2T                    (PE bf16 x fp8)
  dwc          9 bf16 diag-matmul taps over contiguous 462-wide windows
               (bf16 v via a dedicated bf16 v-proj: fp8 noise in the dwc
               chain does not average out and would bust the 2e-2 budget)
  combine      sum = (u * rbc) + (dwc_psum + dwc_b)  (DVE)
  out          proj_w @ sum + proj_b + x             (PE bf16 + DVE)

Per-item tiles are double-buffered and emission is software-pipelined
(head = attention/ACT-heavy, tail = dwc+proj/PE-heavy, interleaved per
channel-group) so consecutive items overlap across engines. Matmuls whose
stationaries load at different PE row bases never share a PSUM bank (HW
restriction found empirically), and each PSUM accumulation chain owns its
2KB zero region.
"""

import sys

for _p in ("/opt/trn_rl_repo", "/opt/trn_rl_repo/concourse"):
    if _p not in sys.path:
        sys.path.insert(0, _p)

import numpy as np
import ml_dtypes

import concourse.bass as bass
import concourse.bacc as bacc
import concourse.mybir as mybir
import concourse.tile as tile
from concourse.bass_utils import run_bass_kernel_spmd

BF16 = ml_dtypes.bfloat16
FP16 = np.float16
F8NP = ml_dtypes.float8_e4m3
FP32 = np.float32


class _Bacc(bacc.Bacc):
    """Bacc whose activation-table pass only sees tables that serve every
    function this kernel uses, so the greedy per-function table choice
    cannot alternate between exp-only and ln-only sets. Table ids keep
    their global act_info.json indices."""

    def insert_act_table_loads(self):
        import concourse.mybir as _mb
        from concourse.hw_specs import get_activation_tables
        import bass_rust as _bass_rust

        acts = [
            i
            for b in self.main_func.blocks
            for i in b.instructions
            if isinstance(i, _mb.InstActivation)
        ]
        if not acts:
            return
        needed = {i.func for i in acts}
        tables = list(get_activation_tables(self.m.arch).items())
        assert any(needed <= funcs for _, funcs in tables), (
            f"no single activation table covers {needed}")
        filtered = [
            (name, funcs if needed <= funcs else set())
            for name, funcs in tables
        ]
        _bass_rust.insert_act_table_loads(self, filtered)


B, C, HH, WW = 16, 256, 56, 56
N = HH * WW            # 3136
NH, HD, A = 8, 32, 49
SCALE = float(HD) ** -0.5
NCORES = 8
BPC = B // NCORES      # 2
CT = 2                 # 128-channel tiles
NP = 25                # ceil(N/128); last chunk is 64
FCH = 448              # free-dim chunk = 8 image rows
NF = 7
PAD = 58
IMG = PAD * PAD        # 3364
VPW = IMG
BD = mybir.dt.bfloat16
FD = mybir.dt.float32
HD16 = mybir.dt.float16
F8 = mybir.dt.float8e4
DRM = mybir.MatmulPerfMode.DoubleRow
FX = mybir.ActivationFunctionType
OP = mybir.AluOpType

# dwc taps (dy, dx), tap index ti = 3*dy + dx, vpad offset = 58*dy + dx.
# All 9 taps run as bf16 diag matmuls over contiguous 462-wide windows
# (fp8 anywhere in the dwc chain costs ~3-4% of the dwc contribution,
# which alone busts the 2e-2 budget); the bias rides the combine's
# scalar_tensor_tensor.
TAPS = [(dy, dx) for dy in range(3) for dx in range(3)]


def _pchunk(i):
    n0 = 128 * i
    return n0, min(128, N - n0)


def _grp(n, size):
    return [list(range(s, min(s + size, n))) for s in range(0, n, size)]


def build_bass(reps=1):
    nc = _Bacc()
    d = {}
    d["xbf"] = nc.declare_dram_parameter("xbf", [BPC, C, N], BD,
                                         isOutput=False)
    d["xq8"] = nc.declare_dram_parameter("xq8", [BPC, C, N], F8,
                                         isOutput=False)
    d["wv8"] = nc.declare_dram_parameter("wv8", [128, 2, C], F8,
                                         isOutput=False)
    d["wkci"] = nc.declare_dram_parameter("wkci", [CT, 128, 2, 128], BD,
                                          isOutput=False)
    d["wqci"] = nc.declare_dram_parameter("wqci", [CT, 128, 2, 128], BD,
                                          isOutput=False)
    d["wprojT"] = nc.declare_dram_parameter("wprojT", [C, C], BD,
                                            isOutput=False)
    d["projb"] = nc.declare_dram_parameter("projb", [C, 1], FD,
                                           isOutput=False)
    d["wvbf"] = nc.declare_dram_parameter("wvbf", [C, C], BD, isOutput=False)
    d["dwcdiag"] = nc.declare_dram_parameter("dwcdiag", [9, CT, 128, 128],
                                             BD, isOutput=False)
    d["dwcb"] = nc.declare_dram_parameter("dwcb", [C, 1], FD, isOutput=False)
    d["bc4"] = nc.declare_dram_parameter("bc4", [128, 128], HD16,
                                         isOutput=False)
    d["out32"] = nc.declare_dram_parameter("out32", [BPC, C, N], BD,
                                           isOutput=True)
    with tile.TileContext(nc) as tc:
        _emit(nc, tc, d, reps)
    nc.finalize()
    return nc


def _emit(nc, tc, d, reps=1):
    import contextlib
    ctx = contextlib.ExitStack()
    with ctx:
        persist = ctx.enter_context(tc.tile_pool(name="persist", bufs=1))
        small = ctx.enter_context(tc.tile_pool(name="small", bufs=2))
        upch = ctx.enter_context(tc.tile_pool(name="upch", bufs=3))
        xbfp = ctx.enter_context(tc.tile_pool(name="xbfp", bufs=2))
        x8p = ctx.enter_context(tc.tile_pool(name="x8p", bufs=2))
        ostp = ctx.enter_context(tc.tile_pool(name="ostp", bufs=2))
        # per-item double-buffered tiles so item k+1's ACT-heavy attention
        # phase can run under item k's PE-heavy dwc/proj phase
        item = ctx.enter_context(tc.tile_pool(name="item", bufs=2))
        # PSUM: "big" 2-bank tiles x2 + "pk" 1-bank tiles x4 = 8 banks
        psum = ctx.enter_context(tc.tile_pool(name="psum", bufs=1,
                                              space="PSUM"))

        t = {}
        t["wv8"] = persist.tile([128, 2, C], F8, name="wv8")
        t["wkci"] = persist.tile([128, CT, 2, 128], BD, name="wkci")
        t["wqci"] = persist.tile([128, CT, 2, 128], BD, name="wqci")
        t["wproj"] = persist.tile([128, 2, C], BD, name="wproj")
        t["projb"] = persist.tile([128, 2, 1], FD, name="projb")
        t["wvbf"] = persist.tile([128, 2, C], BD, name="wvbf")
        t["dwcdiag"] = persist.tile([128, 9, CT, 128], BD, name="dwcdiag")
        t["dwcb"] = persist.tile([128, 2, 1], FD, name="dwcb")
        t["bc4"] = persist.tile([128, 128], HD16, name="bc4")
        t["aT"] = persist.tile([128, CT, A], FD, name="aT")
        t["agBD1"] = persist.tile([128, CT, 4 * A], BD, name="agBD1")
        t["agBD2"] = persist.tile([128, CT, 113], BD, name="agBD2")
        t["onesBD"] = persist.tile([128, 34], BD, name="onesBD")
        t["sum"] = persist.tile([128, CT, N], BD, name="sum")

        nc.vector.memset(t["agBD1"][:, :, :], 0.0)
        nc.vector.memset(t["agBD2"][:, :, :], 0.0)
        # onesBD col 0/32: head-even denom (q=0/1); col 1/33: head-odd;
        # cols 2:32 produce dummy positive sums so one Ln can span rows
        # 0:34 without hitting stale (possibly negative) psum
        nc.vector.memset(t["onesBD"][:, :], 0.0)
        nc.vector.memset(t["onesBD"][0:A, 0:1], 1.0)
        nc.vector.memset(t["onesBD"][0:A, 2:33], 1.0)
        nc.vector.memset(t["onesBD"][64:113, 1:2], 1.0)
        nc.vector.memset(t["onesBD"][64:113, 33:34], 1.0)
        # chunk-24 garbage rows of v_nm/es1 are never read (K=64 there)

        nc.sync.dma_start(out=t["wv8"][:, :, :], in_=d["wv8"][:, :, :])
        nc.sync.dma_start(out=t["wkci"][:, :, :, :],
                          in_=d["wkci"].rearrange("g p c m -> p g c m"))
        nc.sync.dma_start(out=t["wqci"][:, :, :, :],
                          in_=d["wqci"].rearrange("g p c m -> p g c m"))
        nc.scalar.dma_start(out=t["wproj"][:, :, :],
                            in_=d["wprojT"].rearrange("(t p) f -> p t f",
                                                      p=128))
        nc.scalar.dma_start(out=t["projb"][:, :, :],
                            in_=d["projb"].rearrange("(t p) o -> p t o",
                                                     p=128))
        nc.sync.dma_start(out=t["wvbf"][:, :, :],
                          in_=d["wvbf"].rearrange("(t p) f -> p t f", p=128))
        nc.scalar.dma_start(out=t["dwcdiag"][:, :, :, :],
                            in_=d["dwcdiag"].rearrange("k g p f -> p k g f"))
        nc.scalar.dma_start(out=t["dwcb"][:, :, :],
                            in_=d["dwcb"].rearrange("(t p) o -> p t o",
                                                    p=128))
        nc.scalar.dma_start(out=t["bc4"][:, :], in_=d["bc4"][:, :])

        lncp = ctx.enter_context(tc.tile_pool(name="lncp", bufs=2))
        pools = dict(psum=psum, small=small, upch=upch, xbfp=xbfp,
                     x8p=x8p, ostp=ostp, item=item, lncp=lncp,
                     lnc_init=[0])
        idx = 0
        for _ in range(reps):
            for b in range(BPC):
                _emit_item(nc, tc, b, d, t, pools, idx)
                idx += 1


def _big(pools, nm):
    return pools["psum"].tile([128, 1024], FD, name=nm, tag="pbig", bufs=3)


def _bank(pools, nm):
    return pools["psum"].tile([128, 512], FD, name=nm, tag="pk", bufs=2)


DEBUG_PHASE = 99   # 0=loads 1=+pool 2=+preps 3=+s1/v/agv 4=+stage2 5=+dwc


def _dbg_out(nc, b, d, xbf):
    for mt in range(CT):
        nc.gpsimd.dma_start(
            out=d["out32"][b, 128 * mt:128 * (mt + 1), :],
            in_=xbf[:, mt, :])


def _emit_item(nc, tc, b, d, t, pools, idx):
    it = pools["item"]
    t = dict(t)
    t["agK8"] = it.tile([128, 2, 392], F8, name="agK8", tag="agK8", bufs=2)
    t["agQ8"] = it.tile([128, 2, 4, 128], F8, name="agQ8", tag="agQ8",
                        bufs=2)
    t["vpad"] = it.tile([128, CT, VPW], BD, name="vpad", tag="vpad", bufs=2)
    t["v_nm"] = it.tile([128, NP, NH, 33], F8, name="v_nm", tag="v_nm",
                        bufs=2)
    t["es1"] = it.tile([128, NP, NH, 64], F8, name="es1", tag="es1", bufs=2)
    t["w2T"] = it.tile([128, 4, N], F8, name="w2T", tag="w2T", bufs=2)
    t["av_ext"] = it.tile([128, 4, 66], BD, name="av_ext", tag="av_ext",
                          bufs=2)
    t["rtile"] = it.tile([128, 8], FD, name="rtile", tag="rtile", bufs=2)
    t["rbc"] = it.tile([128, CT, N], BD, name="rbc", tag="rbc", bufs=2)
    if idx < 2:
        # per-buffer init of regions later matmuls read but no per-item
        # pass rewrites (each of the two rotating buffers sees this once)
        nc.gpsimd.memset(t["es1"][:, :, :, A:64], 0.0)       # head pads
        nc.gpsimd.memset(t["v_nm"][:, :, :, 32:33], 1.0)     # ones cols
        nc.gpsimd.memset(t["vpad"][:, :, 0:PAD], 0.0)        # top border
        nc.gpsimd.memset(t["vpad"][:, :, 57 * PAD:58 * PAD], 0.0)
        nc.gpsimd.memset(
            t["vpad"][:, :, 0:IMG].rearrange("p g (r c) -> p g r c", c=PAD)
            [:, :, 1:57, 0:1], 0.0)                          # left border
        nc.gpsimd.memset(
            t["vpad"][:, :, 0:IMG].rearrange("p g (r c) -> p g r c", c=PAD)
            [:, :, 1:57, 57:58], 0.0)                        # right border
        nc.vector.memset(t["av_ext"][:, :, :], 0.0)
        nc.vector.memset(t["av_ext"][0:A, :, 64:65], 1.0)    # s2 denoms
        nc.vector.memset(t["av_ext"][64:113, :, 65:66], 1.0)
        nc.gpsimd.memset(t["agQ8"][:, :, :, :], 0.0)   # pad cols 113:128

    # ---------------- load x ----------------
    xbf = pools["xbfp"].tile([128, CT, N], BD, name="xbf", tag="xbf", bufs=2)
    xq8 = pools["x8p"].tile([128, CT, N], F8, name="xq8", tag="x8", bufs=2)
    for g in range(CT):
        nc.sync.dma_start(out=xbf[:, g, :],
                          in_=d["xbf"][b, 128 * g:128 * (g + 1), :])
        nc.scalar.dma_start(out=xq8[:, g, :],
                            in_=d["xq8"][b, 128 * g:128 * (g + 1), :])

    if DEBUG_PHASE <= 0:
        return _dbg_out(nc, b, d, xbf)

    # ------- pooling -> agent blocks (8-way add trees on Pool engine) -----
    for g in range(CT):
        a1 = pools["small"].tile([128, 56, 7], FD, name="a1", tag="a1")
        xv = xbf[:, g, :].rearrange("p (h wb wi) -> p h wb wi", wb=7, wi=8)
        nc.gpsimd.tensor_tensor(out=a1[:, :, :], in0=xv[:, :, :, 0],
                                in1=xv[:, :, :, 1], op=OP.add)
        for k in range(2, 8):
            nc.gpsimd.tensor_tensor(out=a1[:, :, :], in0=a1[:, :, :],
                                    in1=xv[:, :, :, k], op=OP.add)
        a1r = a1[:, :, :].rearrange("p (hb hi) wb -> p hb hi wb", hi=8)
        av = t["aT"][:, g, :].rearrange("p (hb wb) -> p hb wb", wb=7)
        nc.gpsimd.tensor_tensor(out=av, in0=a1r[:, :, 0, :],
                                in1=a1r[:, :, 1, :], op=OP.add)
        for k in range(2, 8):
            nc.gpsimd.tensor_tensor(out=av, in0=av, in1=a1r[:, :, k, :],
                                    op=OP.add)
        # the two 8-sums fold into one /64; scale applied during the copies
        for hp in range(4):
            nc.vector.tensor_scalar_mul(
                out=t["agBD1"][32 * hp:32 * (hp + 1), g, A * hp:A * (hp + 1)],
                in0=t["aT"][32 * hp:32 * (hp + 1), g, :], scalar1=1.0 / 64.0)
        for q in range(2):
            nc.vector.tensor_scalar_mul(
                out=t["agBD2"][64 * q:64 * q + 32, g, 0:A],
                in0=t["aT"][64 * q:64 * q + 32, g, :], scalar1=1.0 / 64.0)
            nc.vector.tensor_scalar_mul(
                out=t["agBD2"][64 * q + 32:64 * q + 64, g, 64:113],
                in0=t["aT"][64 * q + 32:64 * q + 64, g, :], scalar1=1.0 / 64.0)

    if DEBUG_PHASE <= 1:
        return _dbg_out(nc, b, d, xbf)

    # ------------- agentK / agentQ: fold wk, wq into the agents ----------
    # agK[ci, (h,a)] = sum_co wk[co, ci] * agent_bd1[co, (h,a)]
    # (per co-tile g, the 196-wide block of heads 4g..4g+3)
    for c in range(2):
        pk_ = _bank(pools, "pprep")
        for g in range(CT):
            nc.tensor.matmul(pk_[:, 196 * g:196 * (g + 1)],
                             t["wkci"][:, g, c, :], t["agBD1"][:, g, :],
                             start=True, stop=True)
        with nc.allow_low_precision("fp8 agent-key weights"):
            nc.vector.tensor_copy(out=t["agK8"][:, c, :], in_=pk_[:, 0:392])
    # matmuls whose stationaries load at different PE row bases (q=0 vs
    # q=1) must not share a psum bank -> one bank per q
    for c in range(2):
        pqs = [_bank(pools, "pprepA"), _bank(pools, "pprepB")]
        for s in range(4):
            g, q = divmod(s, 2)
            nc.tensor.matmul(pqs[q][:, 128 * g:128 * g + 113],
                             t["wqci"][64 * q:64 * (q + 1), g, c, :],
                             t["agBD2"][64 * q:64 * (q + 1), g, :],
                             start=True, stop=True)
        with nc.allow_low_precision("fp8 agent-query weights"):
            for q in range(2):
                dst = (t["agQ8"][:, c, :, :]
                       .rearrange("p (g qq) m -> p qq g m", qq=2)
                       [:, q, :, 0:113])
                nc.vector.tensor_copy(
                    out=dst,
                    in_=pqs[q][:, 0:256]
                    .rearrange("p (g m) -> p g m", m=128)[:, :, 0:113])

    if DEBUG_PHASE <= 2:
        return _dbg_out(nc, b, d, xbf)

    # ---- stage-1 scores/exp + v_nm + agent_v ----

    def s1_group(iis):
        pt = _big(pools, "p1")
        i0 = iis[0]
        for i in iis:
            n0, sz = _pchunk(i)
            nc.tensor.matmul(
                pt[0:sz, 512 * (i - i0):512 * (i - i0) + 392],
                xq8[:, :, n0:n0 + sz], t["agK8"][:, :, :],
                start=True, stop=True, perf_mode=DRM)
        full = [i for i in iis if _pchunk(i)[1] == 128]
        with nc.allow_low_precision("fp8 attention weights"):
            if full:
                nfull = len(full)
                src = (pt[:, 512 * (full[0] - i0):
                          512 * (full[0] - i0) + 512 * nfull]
                       .rearrange("p (j f) -> p j f", f=512)[:, :, 0:392]
                       .rearrange("p j (h a) -> p j h a", a=A))
                nc.scalar.activation(
                    out=t["es1"][:, full[0]:full[0] + nfull, :, 0:A],
                    in_=src, func=FX.Exp, scale=SCALE)
            for i in iis:
                n0, sz = _pchunk(i)
                if sz == 128:
                    continue
                src = (pt[0:sz, 512 * (i - i0):512 * (i - i0) + 392]
                       .rearrange("p (h a) -> p h a", a=A))
                nc.scalar.activation(
                    out=t["es1"][0:sz, i, :, 0:A],
                    in_=src, func=FX.Exp, scale=SCALE)

    def v_group(iis):
        pt = _big(pools, "pv")
        i0, nj = iis[0], len(iis)
        for i in iis:
            n0, sz = _pchunk(i)
            nc.tensor.matmul(
                pt[0:sz, 512 * (i - i0):512 * (i - i0) + C],
                xq8[:, :, n0:n0 + sz], t["wv8"][:, :, :],
                start=True, stop=True, perf_mode=DRM)
        rows = min(_pchunk(i)[1] for i in iis)
        src = (pt[0:rows, 0:512 * nj].rearrange("p (j f) -> p j f", f=512)
               [:, :, 0:C].rearrange("p j (h dd) -> p j h dd", dd=32))
        if (i0 // 2) % 2 == 0:
            nc.scalar.copy(out=t["v_nm"][0:rows, i0:i0 + nj, :, 0:32],
                           in_=src)
        else:
            with nc.allow_low_precision("fp8 values"):
                nc.vector.tensor_copy(
                    out=t["v_nm"][0:rows, i0:i0 + nj, :, 0:32], in_=src)

    def agv_pair(p):
        # one accumulation chain (own psum bank/zero-region) per head-pair;
        # pairs of full chunks ride one DoubleRow matmul, the 64-row tail
        # chunk is a plain fp8 matmul
        bank = _bank(pools, "agv%d" % p)
        for i in range(0, NP - 1, 2):
            nc.tensor.matmul(
                bank[:, 0:66],
                t["es1"][:, i:i + 2, 2 * p:2 * p + 2, :].rearrange(
                    "p i h dd -> p i (h dd)"),
                t["v_nm"][:, i:i + 2, 2 * p:2 * p + 2, :].rearrange(
                    "p i h dd -> p i (h dd)"),
                start=(i == 0), stop=False, perf_mode=DRM)
        i, (n0, sz) = NP - 1, _pchunk(NP - 1)
        nc.tensor.matmul(
            bank[:, 0:66],
            t["es1"][0:sz, i, 2 * p:2 * p + 2, :].rearrange(
                "p h dd -> p (h dd)"),
            t["v_nm"][0:sz, i, 2 * p:2 * p + 2, :].rearrange(
                "p h dd -> p (h dd)"),
            start=False, stop=True)
        return bank

    def vpad_group(g, js):
        # bf16 v for the dwc path (fp8 v noise does not average out of the
        # depthwise conv and would alone exceed the error budget)
        pt = _big(pools, "pvp")
        j0, nj = js[0], len(js)
        for j in js:
            for kt in range(CT):
                nc.tensor.matmul(
                    pt[:, 512 * (j - j0):512 * (j - j0) + FCH],
                    t["wvbf"][:, kt, 128 * g:128 * (g + 1)],
                    xbf[:, kt, FCH * j:FCH * (j + 1)],
                    start=(kt == 0), stop=(kt == CT - 1))
        dst = (t["vpad"][:, g, 0:IMG]
               .rearrange("p (r c) -> p r c", c=PAD)
               [:, 1 + 8 * j0:1 + 8 * (j0 + nj), 1:57]
               .rearrange("p (j r) c -> p j r c", r=8))
        src = (pt[:, 0:512 * nj].rearrange("p (j f) -> p j f", f=512)
               [:, :, 0:FCH].rearrange("p j (r c) -> p j r c", c=56))
        with nc.allow_low_precision("bf16 values"):
            nc.vector.tensor_copy(out=dst, in_=src)

    groups = _grp(NP, 2)
    vjobs = [(g, js) for g in range(CT) for js in _grp(NF, 2)]
    for k, iis in enumerate(groups):
        s1_group(iis)
        v_group(iis)
        if k < len(vjobs):
            vpad_group(*vjobs[k])
    for g, js in vjobs[len(groups):]:
        vpad_group(g, js)

    for p in range(4):
        bank = agv_pair(p)
        nc.vector.reciprocal(out=t["rtile"][0:A, p:p + 1],
                             in_=bank[0:A, 32:33])
        nc.vector.reciprocal(out=t["rtile"][64:113, 4 + p:5 + p],
                             in_=bank[64:113, 65:66])
        with nc.allow_low_precision("bf16 attention weights"):
            nc.vector.tensor_tensor(
                out=t["av_ext"][0:A, p, 0:32], in0=bank[0:A, 0:32],
                in1=t["rtile"][0:A, p:p + 1].to_broadcast([A, 32]),
                op=OP.mult)
            nc.vector.tensor_tensor(
                out=t["av_ext"][64:113, p, 32:64], in0=bank[64:113, 33:65],
                in1=t["rtile"][64:113, 4 + p:5 + p].to_broadcast([A, 32]),
                op=OP.mult)

    if DEBUG_PHASE <= 3:
        return _dbg_out(nc, b, d, xbf)

    # ------- stage 2: scores/exp + denominator chains ------
    for s in range(4):
        for js in _grp(NF, 2):
            j0, nj = js[0], len(js)
            pt = _big(pools, "p2")
            for j in js:
                nc.tensor.matmul(
                    pt[:, 512 * (j - j0):512 * (j - j0) + FCH],
                    t["agQ8"][:, :, s, :],
                    xq8[:, :, FCH * j:FCH * (j + 1)],
                    start=True, stop=True, perf_mode=DRM)
            src = (pt[0:113, 0:512 * nj]
                   .rearrange("p (j f) -> p j f", f=512)[:, :, 0:FCH])
            dst = (t["w2T"][0:113, s, :]
                   .rearrange("p (j f) -> p j f", f=FCH)
                   [:, j0:j0 + nj, :])
            nc.scalar.activation(out=dst, in_=src, func=FX.Exp,
                                 scale=SCALE)
    for a in range(2):
        # denominators: two ones-matmuls per chunk put head 4a+2q+e's
        # denominator at psum row 32q+e (rows 2:32 get dummy positive sums);
        # ONE Ln per chunk-group covers all four heads; bc4 broadcasts each
        # lns2 row to its head's 32 channel rows; Exp(-x) = 1/s2.
        for js in _grp(NF, 2):
            j0, nj = js[0], len(js)
            pt = _big(pools, "po")
            for j in js:
                nc.tensor.matmul(
                    pt[0:32, 512 * (j - j0):512 * (j - j0) + FCH],
                    t["onesBD"][0:113, 0:32],
                    t["w2T"][0:113, 2 * a, FCH * j:FCH * (j + 1)],
                    start=True, stop=True)
                nc.tensor.matmul(
                    pt[32:34, 512 * (j - j0):512 * (j - j0) + FCH],
                    t["onesBD"][0:113, 32:34],
                    t["w2T"][0:113, 2 * a + 1, FCH * j:FCH * (j + 1)],
                    start=True, stop=True)
            lnc = pools["lncp"].tile([128, 2, FCH], HD16, name="lnc",
                                     tag="lnc", bufs=2)
            src = (pt[0:34, 0:512 * nj]
                   .rearrange("p (j f) -> p j f", f=512)[:, :, 0:FCH])
            with nc.allow_low_precision("fp16 log-denominators"):
                nc.scalar.activation(out=lnc[0:34, 0:nj, :], in_=src,
                                     func=FX.Ln)
            pb = _big(pools, "pb")
            for j in js:
                nc.tensor.matmul(
                    pb[:, 512 * (j - j0):512 * (j - j0) + FCH],
                    t["bc4"][0:34, :], lnc[0:34, j - j0, :],
                    start=True, stop=True)
            bsrc = (pb[:, 0:512 * nj].rearrange("p (j f) -> p j f", f=512)
                    [:, :, 0:FCH])
            dst = (t["rbc"][:, a, :].rearrange("p (j f) -> p j f", f=FCH)
                   [:, j0:j0 + nj, :])
            with nc.allow_low_precision("bf16 softmax reciprocal"):
                nc.scalar.activation(out=dst, in_=bsrc, func=FX.Exp,
                                     scale=-1.0)

    if DEBUG_PHASE <= 4:
        return _dbg_out(nc, b, d, xbf)

    # ---------------- dwc + apply + combine ----------------
    # dwc: 5 fp8-DR diag-pair matmuls per (g, j); the last pair is
    # (tap 8, bias) with its second k-tile reading vpad's ones region.
    # moving operand per (tap, j): contiguous 462-wide window over 8 padded
    # rows starting at the tap offset. Output position f = 58*r + c maps to
    # image pixel (8j+r, c); f % 58 in {56, 57} is junk never read back.
    for g in range(2):
        vpg = t["vpad"][:, g, :]
        for half in _grp(NF, 2):
            pds = [_big(pools, "pd") for _ in range((len(half) + 1) // 2)]
            for k in range(9):
                dy, dx = TAPS[k]
                for idx, j in enumerate(half):
                    off1 = 58 * (dy + 8 * j) + dx
                    mv = bass.AP(
                        tensor=vpg.tensor,
                        offset=vpg.offset + off1,
                        ap=[[CT * VPW, 128], [1, 462]])
                    nc.tensor.matmul(
                        pds[idx // 2][:, 512 * (idx % 2):512 * (idx % 2)
                                      + 462],
                        t["dwcdiag"][:, k, g, :], mv,
                        start=(k == 0), stop=(k == 8))
            pas = [_bank(pools, "pa") for _ in half]
            for q in range(2):
                for idx, j in enumerate(half):
                    nc.tensor.matmul(
                        pas[idx][64 * q:64 * (q + 1), 0:FCH],
                        t["av_ext"][0:113, 2 * g + q, 0:64],
                        t["w2T"][0:113, 2 * g + q, FCH * j:FCH * (j + 1)],
                        start=True, stop=True)
            for idx, j in enumerate(half):
                pd_ap = pds[idx // 2][:, :]
                pd = bass.AP(tensor=pd_ap.tensor,
                             offset=pd_ap.offset + 512 * (idx % 2),
                             ap=[[1024, 128], [58, 8], [1, 56]])
                pa = pas[idx]
                upt = pools["upch"].tile([128, FCH], BD, name="up", tag="up")
                with nc.allow_low_precision("bf16 activations"):
                    # u' = u * (1/s2)          (DVE, PSUM read)
                    nc.vector.tensor_tensor(
                        out=upt[:, :], in0=pa[:, 0:FCH],
                        in1=t["rbc"][:, g, FCH * j:FCH * (j + 1)], op=OP.mult)
                    # sum = (dwc_psum + dwc_b) + u'   (DVE, PSUM read)
                    nc.vector.scalar_tensor_tensor(
                        out=t["sum"][:, g, FCH * j:FCH * (j + 1)]
                        .rearrange("p (r c) -> p r c", c=56),
                        in0=pd, scalar=t["dwcb"][:, g, :],
                        in1=upt[:, :].rearrange("p (r c) -> p r c", c=56),
                        op0=OP.add, op1=OP.add)

    if DEBUG_PHASE <= 5:
        return _dbg_out(nc, b, d, xbf)

    # ---------------- proj + bias + residual ----------------
    for mt in range(CT):
        for js in _grp(NF, 2):
            j0, nj = js[0], len(js)
            pp = _big(pools, "pp")
            for kt in range(CT):
                for j in js:
                    nc.tensor.matmul(
                        pp[:, 512 * (j - j0):512 * (j - j0) + FCH],
                        t["wproj"][:, kt, 128 * mt:128 * (mt + 1)],
                        t["sum"][:, kt, FCH * j:FCH * (j + 1)],
                        start=(kt == 0), stop=(kt == CT - 1))
            src = (pp[:, 0:512 * nj].rearrange("p (j f) -> p j f", f=512)
                   [:, :, 0:FCH])
            ost = pools["ostp"].tile([128, 2, FCH], BD, name="ost",
                                     tag="ost", bufs=2)
            resid = (xbf[:, mt, FCH * j0:FCH * (j0 + nj)]
                     .rearrange("p (j f) -> p j f", f=FCH))
            with nc.allow_low_precision("bf16 output"):
                nc.vector.scalar_tensor_tensor(
                    out=ost[:, 0:nj, :], in0=src, scalar=t["projb"][:, mt, :],
                    in1=resid, op0=OP.add, op1=OP.add)
            nc.gpsimd.dma_start(
                out=d["out32"][b, 128 * mt:128 * (mt + 1),
                               FCH * j0:FCH * (j0 + nj)],
                in_=ost[:, 0:nj, :])


def host_prep(x, qkv_w, proj_w, proj_b, dwc_w, dwc_b):
    xf = np.ascontiguousarray(x.reshape(B, C, N), dtype=FP32)
    qkv = np.asarray(qkv_w, FP32)
    wv8 = np.zeros((128, 2, C), FP32)
    for kt in range(2):
        # wv8[p, kt, co] = qkv_w[512+co, 128*kt+p]
        wv8[:, kt, :] = qkv[512:768, 128 * kt:128 * (kt + 1)].T
    wv8 = np.ascontiguousarray(wv8.astype(F8NP))
    wkci = np.zeros((CT, 128, 2, 128), FP32)
    wqci = np.zeros((CT, 128, 2, 128), FP32)
    for g in range(CT):
        for c in range(2):
            # wkci[g, p, c, m] = wk[128g+p, 128c+m]
            wkci[g, :, c, :] = qkv[256 + 128 * g:256 + 128 * (g + 1),
                                   128 * c:128 * (c + 1)]
            # wqci[g, 64q+p, c, m] = wq[64*(2g+q)+p, 128c+m]
            for q in range(2):
                s = 2 * g + q
                wqci[g, 64 * q:64 * (q + 1), c, :] = (
                    qkv[64 * s:64 * (s + 1), 128 * c:128 * (c + 1)])
    wkci = np.ascontiguousarray(wkci.astype(BF16))
    wqci = np.ascontiguousarray(wqci.astype(BF16))
    wprojT = np.ascontiguousarray(np.asarray(proj_w, FP32).T.astype(BF16))
    wvbfT = np.ascontiguousarray(qkv[512:768].T.astype(BF16))
    projb = np.ascontiguousarray(np.asarray(proj_b, FP32).reshape(C, 1))
    dwcb = np.ascontiguousarray(np.asarray(dwc_b, FP32).reshape(C, 1))
    w33 = np.asarray(dwc_w, FP32).reshape(C, 9)
    dd = np.zeros((9, CT, 128, 128), FP32)
    for k in range(9):
        for g in range(CT):
            np.fill_diagonal(dd[k, g], w33[128 * g:128 * (g + 1), k])
    dwcdiag = np.ascontiguousarray(dd.astype(BF16))
    bc4 = np.zeros((128, 128), FP32)
    for e, row in enumerate((0, 1, 32, 33)):
        bc4[row, 32 * e:32 * (e + 1)] = 1.0
    bc4 = np.ascontiguousarray(bc4.astype(FP16))
    maps = []
    for c in range(NCORES):
        xs = xf[BPC * c:BPC * (c + 1)]
        maps.append(dict(
            xbf=np.ascontiguousarray(xs.astype(BF16)),
            xq8=np.ascontiguousarray(xs.astype(F8NP)),
            wv8=wv8, wkci=wkci, wqci=wqci, wprojT=wprojT, wvbf=wvbfT,
            projb=projb, dwcdiag=dwcdiag, dwcb=dwcb, bc4=bc4,
        ))
    return maps


_NC_CACHE = {}


def kernel(x, qkv_w, proj_w, proj_b, dwc_w, dwc_b, trace=False):
    if "nc" not in _NC_CACHE:
        _NC_CACHE["nc"] = build_bass()
    nc = _NC_CACHE["nc"]
    maps = host_prep(np.asarray(x), np.asarray(qkv_w), np.asarray(proj_w),
                     np.asarray(proj_b), np.asarray(dwc_w), np.asarray(dwc_b))
    res = run_bass_kernel_spmd(nc, maps, core_ids=list(range(NCORES)),
                               trace=trace)
    outs = [np.asarray(r["out32"]).astype(np.float32).reshape(BPC, C, HH, WW)
            for r in res.results]
    full = np.concatenate(outs, axis=0)
    if trace:
        return full, res
    return full
